# revision 9
# baseline (speedup 1.0000x reference)
"""Trainium2 Bass kernel for nn_MemoryGame (scatter_memory).

Strategy (8 NeuronCores, tensor-parallel over M's columns):
  - Tiny MLPs / outer products / readout run on host (microseconds of work).
  - The heavy part (50-iteration Hopfield loop over M [9216,9216] plus the
    rank-1 Hebbian update of M) runs on 8 cores.
  - M's columns are sharded interleaved: core c owns columns j with
    (j % 128)//16 == c, stored locally in (p', f) order where the global
    column is j = f*128 + 16c + p'.  With this order, the per-iteration
    AllGather of the 8 local h-slices concatenates into exactly the
    partition-major [128, 72] layout the TensorEngine needs for the next
    matvec -- every DMA stays contiguous.
  - M is cast to fp16 and kept resident in SBUF (21.2MB/core): HBM traffic is
    one 42.5MB read (load) and one 42.5MB write (M_out) per core, total.
  - h stays f32 everywhere (it decays to ~1e-5, below fp16-normal range);
    only the matmul stationary operand is a per-iteration fp16 cast of h.
"""

import numpy as np

P = 9216
DIM_X = 96
DIM_G = 96
NUM_CLASS = 1000
N_ITER = 50
KAPPA, LAMDA, YITA = 0.8, 0.9, 0.1
LEAKY_SLOPE = 0.01

NCORES = 8
LOCAL = P // NCORES          # 1152 columns per core
F = P // 128                 # 72 k-chunks / free size of pm layout
PF = 128 // NCORES           # 16 partitions per core in the interleave
NSPLITS = [(0, 512), (512, 512), (1024, 128)]  # LOCAL split into <=512 psum tiles

_BUILD_CACHE = {}


def _build_nc(n_iter=N_ITER, use_cc=True):
    import concourse.bacc as bacc
    import concourse.mybir as mybir
    import concourse.tile as tile

    f32 = mybir.dt.float32
    f16 = mybir.dt.float16
    Alu = mybir.AluOpType

    nc = bacc.Bacc("TRN2", target_bir_lowering=False, num_devices=NCORES)

    m_in = nc.dram_tensor("m_in", [P, LOCAL], f32, kind="ExternalInput")
    h0pm = nc.dram_tensor("h0pm", [128, F], f32, kind="ExternalInput")
    h0loc = nc.dram_tensor("h0loc", [1, LOCAL], f32, kind="ExternalInput")
    ppm = nc.dram_tensor("ppm", [128, F], f32, kind="ExternalInput")
    ploc = nc.dram_tensor("ploc", [1, LOCAL], f32, kind="ExternalInput")
    pout = nc.dram_tensor("pout", [1, LOCAL], f32, kind="ExternalOutput")
    mout = nc.dram_tensor("mout", [P, LOCAL], f32, kind="ExternalOutput")

    rg = [list(range(NCORES))]

    with tile.TileContext(nc) as tc:
        with (
            tc.tile_pool(name="mpool", bufs=1) as mpool,
            tc.tile_pool(name="stage", bufs=2) as stpool,
            tc.tile_pool(name="small", bufs=1) as sm,
            tc.tile_pool(name="rot", bufs=2) as rot,
            tc.tile_pool(name="psum", bufs=1, space="PSUM") as ps,
            tc.tile_pool(name="dram", bufs=n_iter, space="DRAM") as dr,
        ):
            M16 = mpool.tile([128, F * LOCAL], f16)

            # ---- Phase A: load M shard, cast f32 -> fp16 into SBUF ----
            for t in range(F):
                stg = stpool.tile([128, LOCAL], f32, tag="stage")
                nc.sync.dma_start(stg[:], m_in[t * 128:(t + 1) * 128, :])
                nc.vector.tensor_copy(M16[:, t * LOCAL:(t + 1) * LOCAL], stg[:])

            # ---- persistent small tiles ----
            p_pm = sm.tile([128, F], f32, tag="p_pm")
            nc.sync.dma_start(p_pm[:], ppm[:])
            p_loc = sm.tile([1, LOCAL], f32, tag="p_loc")
            nc.sync.dma_start(p_loc[:], ploc[:])
            h16 = sm.tile([128, F], f16, tag="h16")
            u = sm.tile([1, LOCAL], f32, tag="u")
            v = sm.tile([1, LOCAL], f32, tag="v")

            h_pm = rot.tile([128, F], f32, tag="hpm")
            nc.sync.dma_start(h_pm[:], h0pm[:])
            h_loc = rot.tile([1, LOCAL], f32, tag="hloc")
            nc.sync.dma_start(h_loc[:], h0loc[:])

            acc = [
                ps.tile([1, 512], f32, tag="acc0", name="acc0"),
                ps.tile([1, 512], f32, tag="acc1", name="acc1"),
                ps.tile([1, 128], f32, tag="acc2", name="acc2"),
            ]

            # ---- Phase B: Hopfield attractor loop ----
            for it in range(n_iter):
                nc.vector.tensor_copy(h16[:], h_pm[:])  # f32 -> fp16 cast
                for t in range(F):
                    lhsT = h16[:, t:t + 1]
                    for a, (off, sz) in zip(acc, NSPLITS):
                        nc.tensor.matmul(
                            a[:],
                            lhsT,
                            M16[:, t * LOCAL + off: t * LOCAL + off + sz],
                            start=(t == 0),
                            stop=(t == F - 1),
                        )
                # u = (hM + kappa) * h   (piecewise over the psum tiles)
                for a, (off, sz) in zip(acc, NSPLITS):
                    nc.vector.scalar_tensor_tensor(
                        u[:, off:off + sz], a[:], KAPPA, h_loc[:, off:off + sz],
                        Alu.add, Alu.mult,
                    )
                # leaky relu: max(u, 0.01*u); then clamp to [-1, 1]
                nc.vector.tensor_scalar_mul(v[:], u[:], LEAKY_SLOPE)
                nc.vector.tensor_tensor(v[:], u[:], v[:], Alu.max)
                h_loc_new = rot.tile([1, LOCAL], f32, tag="hloc")
                nc.vector.tensor_scalar(
                    h_loc_new[:], v[:], 1.0, -1.0, Alu.min, Alu.max
                )

                # all-gather the 8 local slices into pm order
                bin_ = dr.tile([1, LOCAL], f32, tag="bin")
                bout = dr.tile([NCORES, LOCAL], f32, tag="bout")
                nc.sync.dma_start(bin_[:], h_loc_new[:])
                if use_cc:
                    nc.gpsimd.collective_compute(
                        "AllGather", Alu.bypass, replica_groups=rg,
                        ins=[bin_[:].opt()], outs=[bout[:].opt()],
                    )
                else:  # debug: fake gather (wrong data, same dataflow)
                    for r in range(NCORES):
                        nc.sync.dma_start(bout[r:r + 1, :], bin_[:])
                h_pm_new = rot.tile([128, F], f32, tag="hpm")
                nc.sync.dma_start(
                    h_pm_new[:], bout[:].rearrange("a (b c) -> (a b) c", b=PF)
                )
                h_loc, h_pm = h_loc_new, h_pm_new

            # ---- outputs: p_ slice ----
            nc.sync.dma_start(pout[:], h_loc[:])

            # ---- Phase D: M_out = 0.9*M + 0.1*(p+p_)(p-p_)^T (column shard) ----
            a_pm = sm.tile([128, F], f32, tag="apm")
            nc.vector.tensor_add(a_pm[:], p_pm[:], h_pm[:])
            # bdiff = p_loc - h_loc, computed in place into p_loc
            nc.vector.tensor_sub(p_loc[:], p_loc[:], h_loc[:])
            # broadcast 0.1*bdiff across 128 partitions via K=1 outer product
            oneY = sm.tile([1, 128], f32, tag="oneY")
            nc.vector.memset(oneY[:], YITA)
            B01 = sm.tile([128, LOCAL], f32, tag="B01")
            for (off, sz), tg in zip(NSPLITS, ["bb0", "bb1", "bb2"]):
                bb = ps.tile([128, sz], f32, tag=tg, name=tg)
                nc.tensor.matmul(
                    bb[:], oneY[:], p_loc[:, off:off + sz], start=True, stop=True
                )
                nc.vector.tensor_copy(B01[:, off:off + sz], bb[:])
            for t in range(F):
                ot = stpool.tile([128, LOCAL], f32, tag="stage")
                nc.vector.tensor_scalar_mul(ot[:], B01[:], a_pm[:, t:t + 1])
                nc.vector.scalar_tensor_tensor(
                    ot[:], M16[:, t * LOCAL:(t + 1) * LOCAL], LAMDA, ot[:],
                    Alu.mult, Alu.add,
                )
                nc.sync.dma_start(mout[t * 128:(t + 1) * 128, :], ot[:])

    nc.compile()
    return nc


def _get_nc():
    if "nc" not in _BUILD_CACHE:
        _BUILD_CACHE["nc"] = _build_nc()
    return _BUILD_CACHE["nc"]


def _col_idx(c):
    """Global column indices owned by core c, in local (p'-major) order."""
    J = np.arange(P, dtype=np.int64).reshape(F, 128)  # J[f, p] = f*128 + p
    return J[:, PF * c: PF * (c + 1)].T.reshape(-1)   # (p', f) -> f*128+16c+p'


def _pm(vec):
    """[P] vector -> partition-major [128, F]: out[p, f] = vec[f*128 + p]."""
    return np.ascontiguousarray(vec.reshape(F, 128).T)


def kernel(x, g, M, W1x, b1x, W2x, b2x, W1g, b1g, W2g, b2g, Wp, bp):
    from concourse.bass_utils import run_bass_kernel_spmd

    f32 = np.float32
    x = np.asarray(x, f32); g = np.asarray(g, f32); M = np.asarray(M, f32)

    # ---- host: tiny MLPs, p, query, h0 ----
    def mlp2(inp, W1, b1, W2, b2):
        h = inp @ np.asarray(W1, f32) + np.asarray(b1, f32)
        return np.maximum(h, 0.0) @ np.asarray(W2, f32) + np.asarray(b2, f32)

    x_ = mlp2(x, W1x, b1x, W2x, b2x)[0]   # [96]
    g_ = mlp2(g, W1g, b1g, W2g, b2g)[0]   # [96]
    p = np.outer(x_, g_).reshape(-1).astype(f32)       # [9216]
    query = np.tile(g_, DIM_X).astype(f32)             # [9216]
    h0 = np.clip(np.maximum(query, LEAKY_SLOPE * query), -1.0, 1.0).astype(f32)

    M0 = M.reshape(P, P)
    arr = M0.reshape(P, F, 128)
    h0_pm = _pm(h0)
    p_pm = _pm(p)

    in_maps = []
    for c in range(NCORES):
        shard = np.ascontiguousarray(
            arr[:, :, PF * c: PF * (c + 1)].transpose(0, 2, 1).reshape(P, LOCAL)
        )
        h0_loc = np.ascontiguousarray(h0_pm[PF * c: PF * (c + 1), :]).reshape(1, LOCAL)
        p_loc = np.ascontiguousarray(p_pm[PF * c: PF * (c + 1), :]).reshape(1, LOCAL)
        in_maps.append({
            "m_in": shard,
            "h0pm": h0_pm,
            "h0loc": h0_loc,
            "ppm": p_pm,
            "ploc": p_loc,
        })

    nc = _get_nc()
    res = run_bass_kernel_spmd(nc, in_maps, core_ids=list(range(NCORES)))
    results = res.results

    # ---- host: reassemble ----
    p_pm_full = np.empty((128, F), f32)
    R = np.empty((P, F, 128), f32)
    for c in range(NCORES):
        p_pm_full[PF * c: PF * (c + 1), :] = results[c]["pout"].reshape(PF, F)
        R[:, :, PF * c: PF * (c + 1)] = (
            results[c]["mout"].reshape(P, PF, F).transpose(0, 2, 1)
        )
    p_ = np.ascontiguousarray(p_pm_full.T).reshape(1, P)
    M_out = R.reshape(1, P, P)

    x_out = p_.reshape(DIM_X, DIM_G).sum(axis=1)
    x_inf = (x_out @ np.asarray(Wp, f32) + np.asarray(bp, f32)).astype(f32)

    return x_inf, p_, M_out
